# revision 16
# baseline (speedup 1.0000x reference)
"""Trainium2 Bass kernel for CompanySpecificHeads (MoE-style routed MLP heads).

Semantics (matching the reference):
    out[b] = gelu(z[b] @ W1[cid[b]] + b1[cid[b]]) @ W2[cid[b]] + b2[cid[b]]

Strategy: expert-parallel across 8 NeuronCores, 8 companies per core.
The run sits on the roofline ridge: per core ~4.9MB of input DMA (HBM cap
~358 GB/s shared by both HWDGE rings) against ~1.6us/slot of PE work.
W1 is carried in fp8 E3M4 (scaled x64 on host; the x1/64 descale is
folded into the gelu activation's `scale`) while z / b1 / W2 / gelu
output stay fp16 -> mixed fp8xfp16 matmuls (verified exact on HW).

Token capacity is variable per company slot: companies are sorted by
token count (descending) and slot s takes ranks [8s, 8s+8) across the 8
cores, so the shared SPMD program pads only to the group max (~552
padded tokens/core vs 896 uniform). Per company (tokens on the free
axis, h on partitions):
  L1:   psum[h, t] = sum_d W1[c][d, h] * zT[c][d, t]   (PE, fp8 x fp16)
  bias: psum += 64*b1[c][h]  broadcast along t          (DVE, stride-0 AP)
  Gelu: ht = gelu(psum/64)   psum -> SBUF fp16          (ACT)
  L2:   psum2[1, t] += W2[c][hj]^T @ ht[hj, t]          (PE, 8 matmuls)
Host scatters back to [B, 1] and adds b2 (exact, fp32).

Trace-driven scheduling (v5):
- DMAs with small per-partition runs (b1 128B/p, z-slices 700B/p
  strided) crawl at ~45GB/s AND stall the HWDGE ring FIFO behind them.
  All head data (b1 | W2 | z[slot0] | W1[slot0,g0]) is byte-packed by
  the host into ONE contiguous [128, 3008B] uint8 tensor, DMA'd as the
  ring's first transfer at full rate, and sliced on-chip via bitcast
  views. Every z chunk gets its own contiguous dram tensor + tile.
- W1 striped across both HWDGE rings in compute order (slot0 g1 is its
  own transfer right behind the head block).
- No bias matmuls / selector: bias is a DVE tensor_add into PSUM with a
  [128,KC,tw] stride-0 broadcast AP (saves ~0.5us/slot of PE bubbles).
- Layer 2 + output copy of slot s are emitted AFTER layer 1 of slot
  s+1 (software pipelining) so the PE never waits on the gelu.
- Warmup is 20 cheap N=128 matmuls that drain right as the head lands.
- Output stores: one per ring, in the tail shadow.
"""

import numpy as np

B, C, D, H = 4096, 64, 512, 1024
NCORES = 8
CPC = C // NCORES  # companies (slots) per core
KC = D // 128      # contraction chunks of 128
HC = H // 128      # h chunks of 128
WSCALE = 64.0      # W1 fp8 scale (descaled in the gelu activation)
F8MAX = 15.5       # e3m4 max finite

_COMPILED = {}


def _build(caps):
    """Build the Bass/Tile program for per-slot token capacities `caps`."""
    import concourse.bacc as bacc
    import concourse.mybir as mybir
    from concourse.tile import TileContext
    from contextlib import ExitStack

    f32 = mybir.dt.float32
    f16 = mybir.dt.float16
    f8 = mybir.dt.float8e3
    u8 = mybir.dt.uint8

    ooff = [0]
    for cap in caps:
        ooff.append(ooff[-1] + cap)
    OUTCOLS = ooff[-1]

    # Byte layout of the packed head block, per partition:
    #   [b1 64xf16][w2 64xf16][z0 KC*cap0 xf16][w1 slot0 g0 2048xf8]
    B1B = CPC * 2 * KC * 2          # 128 bytes
    W2B = CPC * HC * 2              # 128 bytes
    Z0B = KC * caps[0] * 2
    W1GB = 2 * KC * (H // 2)        # 4096 bytes (both g halves, fp8)
    HEADB = B1B + W2B + Z0B + W1GB

    nc = bacc.Bacc(None, target_bir_lowering=False)

    head_d = nc.dram_tensor("head", [128, HEADB], u8, kind="ExternalInput")
    # w1[s][p][g][k][hh] = W1[comp(s)][128k+p, 512g+hh] * 64 for s >= 1
    w1_d = nc.dram_tensor(
        "w1", [CPC - 1, 128, 2, KC, H // 2], f8, kind="ExternalInput"
    )
    # z chunks (partition-major, contiguous): zb = slots 1-3, zc = slots 4-7
    ZB = sum(KC * c for c in caps[1:4])
    ZC = sum(KC * c for c in caps[4:])
    zb_d = nc.dram_tensor("zb", [128, ZB], f16, kind="ExternalInput")
    zc_d = nc.dram_tensor("zc", [128, ZC], f16, kind="ExternalInput")
    out_d = nc.dram_tensor("out", [1, OUTCOLS], f32, kind="ExternalOutput")

    gelu = mybir.ActivationFunctionType.Gelu

    with TileContext(nc) as tc, ExitStack() as ctx:
        const = ctx.enter_context(tc.tile_pool(name="const", bufs=1))
        headt = const.tile([128, HEADB], u8)
        b1p = headt[:, 0:B1B].bitcast(f16)
        w2t = headt[:, B1B:B1B + W2B].bitcast(f16)
        z0v = headt[:, B1B + W2B:B1B + W2B + Z0B].bitcast(f16)
        w10v = headt[:, B1B + W2B + Z0B:HEADB].bitcast(f8).rearrange(
            "p (g k h) -> p g k h", g=2, k=KC
        )
        zbt = const.tile([128, ZB], f16)
        zct = const.tile([128, ZC], f16)

        w1p = ctx.enter_context(tc.tile_pool(name="w1p", bufs=1))
        w1ts = [
            w1p.tile([128, 2, KC, H // 2], f8, name=f"w1_{s}")
            for s in range(1, CPC)
        ]

        # SP ring: packed head first (full-rate), then slot0 g1 and even
        # W1 slots. ACT ring (starts ~0.7us later after its table-load
        # preamble): odd W1 slots interleaved with the z chunks.
        nc.sync.dma_start(out=headt[:], in_=head_d[:])
        nc.scalar.dma_start(out=zbt[:], in_=zb_d[:])
        nc.scalar.dma_start(out=w1ts[0][:], in_=w1_d[0])
        nc.sync.dma_start(out=w1ts[1][:], in_=w1_d[1])
        nc.scalar.dma_start(out=w1ts[2][:], in_=w1_d[2])
        nc.sync.dma_start(out=w1ts[3][:], in_=w1_d[3])
        nc.scalar.dma_start(out=zct[:], in_=zc_d[:])
        nc.scalar.dma_start(out=w1ts[4][:], in_=w1_d[4])
        nc.sync.dma_start(out=w1ts[5][:], in_=w1_d[5])
        nc.sync.dma_start(out=w1ts[6][:], in_=w1_d[6])

        # Staged per-company outputs; stored at the end, one per ring.
        oall = const.tile([1, OUTCOLS], f32)

        hp = ctx.enter_context(tc.tile_pool(name="hp", bufs=16))
        pp = ctx.enter_context(tc.tile_pool(name="pp", bufs=6, space="PSUM"))
        opp = ctx.enter_context(tc.tile_pool(name="opp", bufs=2, space="PSUM"))

        # PE warmup: cheap matmuls keep the PE's HAM activity window busy
        # from the end of the framework preamble (~6us) until the head
        # block lands (~9.7us) so real matmuls start at the warm clock.
        wsc = const.tile([128, 128], f16)
        nc.vector.memset(wsc[:], 0.0)
        wp_t = pp.tile([128, KC * 128], f32, name="ps")
        wp = wp_t[:, :128]
        for _ in range(12):
            nc.tensor.matmul(wp, wsc[:], wsc[:], start=True, stop=True)

        # z-chunk view + column offset for a slot
        def z_of(s):
            if s == 0:
                return z0v, 0
            if s < 4:
                return zbt, sum(KC * c for c in caps[1:s])
            return zct, sum(KC * c for c in caps[4:s])

        def w1_of(s, g):
            if s == 0:
                return w10v[:, g]
            return w1ts[s - 1][:, g]

        # Software-pipelined main loop: layer 2 of iteration i-1 is
        # emitted after layer 1 of iteration i, so the PE never waits on
        # the gelu. PSUM: 2 ps banks/iter x 2 iters in flight + osum.
        pending = None  # (s, t0, tw, [ht_g0, ht_g1])

        def emit_layer2(item):
            s, t0, tw, hts = item
            osum_t = opp.tile([1, 128], f32, name="osum")
            osum = osum_t[:, :tw]
            for g in range(2):
                for j in range(KC):
                    jj = KC * g + j
                    nc.tensor.matmul(
                        osum,
                        w2t[:, HC * s + jj:HC * s + jj + 1],
                        hts[g][:, j * tw:(j + 1) * tw],
                        start=(jj == 0),
                        stop=(jj == HC - 1),
                    )
            nc.vector.tensor_copy(oall[:, ooff[s] + t0: ooff[s] + t0 + tw], osum)

        for s in range(CPC):
            cap = caps[s]
            zv, zbase = z_of(s)
            for t0 in range(0, cap, 128):
                tw = min(128, cap - t0)
                hts = []
                for g in range(2):
                    w1g = w1_of(s, g)
                    ps_t = pp.tile([128, KC * 128], f32, name="ps")
                    ps = ps_t[:, :KC * tw]
                    zb = zbase + t0
                    for j in range(KC):
                        for k in range(KC):
                            nc.tensor.matmul(
                                ps[:, j * tw:(j + 1) * tw],
                                w1g[:, k, 128 * j:128 * (j + 1)],
                                zv[:, zb + k * cap: zb + k * cap + tw],
                                start=(k == 0),
                                stop=(k == KC - 1),
                            )
                    # psum += 64*b1 broadcast along tokens (stride-0 AP)
                    col = (s * 2 + g) * KC
                    ht_t = hp.tile([128, KC * 128], f16, name="ht")
                    ht = ht_t[:, :KC * tw]
                    last = s == CPC - 1 and t0 + 128 >= cap and g == 1
                    halves = 2 if last else 1
                    hk = KC // halves
                    for hh in range(halves):
                        pshh = ps[:, hh * hk * tw:(hh + 1) * hk * tw]
                        nc.vector.tensor_add(
                            pshh,
                            pshh,
                            b1p[:, col + hh * hk:col + (hh + 1) * hk]
                            .unsqueeze(-1)
                            .broadcast_to([128, hk, tw]),
                        )
                        nc.scalar.activation(
                            ht[:, hh * hk * tw:(hh + 1) * hk * tw],
                            pshh,
                            gelu,
                            scale=1.0 / WSCALE,
                        )
                    hts.append(ht)
                if pending is not None:
                    emit_layer2(pending)
                pending = (s, t0, tw, hts)
        # Tail: the final slot's layer 2. g0's matmuls are ready (its
        # gelu overlapped g1's layer 1); g1's wait only on the last gelu.
        s, t0, tw, hts = pending
        osum_t = opp.tile([1, 128], f32, name="osum")
        osum = osum_t[:, :tw]
        for g in range(2):
            for j in range(KC):
                jj = KC * g + j
                nc.tensor.matmul(
                    osum,
                    w2t[:, HC * s + jj:HC * s + jj + 1],
                    hts[g][:, j * tw:(j + 1) * tw],
                    start=(jj == 0),
                    stop=(jj == HC - 1),
                )
        nc.vector.tensor_copy(oall[:, ooff[s] + t0: ooff[s] + t0 + tw], osum)

        olast = ooff[max(1, CPC - 1)]
        nc.sync.dma_start(out=out_d[:, :olast], in_=oall[:, :olast])
        nc.scalar.dma_start(out=out_d[:, olast:], in_=oall[:, olast:])

    nc.finalize()
    return nc


def _get_compiled(caps):
    key = tuple(caps)
    if key not in _COMPILED:
        _COMPILED[key] = _build(key)
    return _COMPILED[key]


def kernel(z, company_id, W1, b1, W2, b2):
    import ml_dtypes
    from concourse.bass_utils import run_bass_kernel_spmd

    f8np = ml_dtypes.float8_e3m4
    z = np.asarray(z, dtype=np.float32)
    cid = np.asarray(company_id).astype(np.int64).ravel()
    W1 = np.asarray(W1, dtype=np.float32)
    b1 = np.asarray(b1, dtype=np.float32)
    W2 = np.asarray(W2, dtype=np.float32)
    b2 = np.asarray(b2, dtype=np.float32)
    O = W2.shape[2]

    idx_by_company = [np.nonzero(cid == gc)[0] for gc in range(C)]
    cnt = np.array([len(ix) for ix in idx_by_company])
    order = np.argsort(-cnt, kind="stable")  # descending token count
    # slot s <- ranks [8s, 8s+8): core i takes order[8s+i]; shared capacity
    # is the group max rounded to 8.
    comp_at = [[int(order[CPC * s + core]) for s in range(CPC)] for core in range(NCORES)]
    caps = tuple(
        max(8, int(np.ceil(cnt[order[CPC * s]] / 8)) * 8) for s in range(CPC)
    )

    nc = _get_compiled(caps)

    ooffs = np.concatenate([[0], np.cumsum(caps)])
    OUTCOLS = int(ooffs[-1])

    def zchunk(core, slots):
        """[128, sum KC*cap] partition-major routed tokens for `slots`."""
        cols = []
        for s in slots:
            gc = comp_at[core][s]
            cap = caps[s]
            ix = idx_by_company[gc]
            zslot = np.zeros((cap, D), dtype=np.float16)
            if len(ix):
                zslot[:len(ix)] = z[ix].astype(np.float16)
            cols.append(
                zslot.reshape(cap, KC, 128).transpose(2, 1, 0).reshape(128, KC * cap)
            )
        return np.concatenate(cols, axis=1)

    def w1block(core, s):
        gc = comp_at[core][s]
        return (
            np.clip(W1[gc] * WSCALE, -F8MAX, F8MAX)
            .reshape(KC, 128, 2, H // 2)
            .transpose(1, 2, 0, 3)
            .astype(f8np)
        )

    in_maps = []
    for core in range(NCORES):
        b1p = np.zeros((128, CPC * 2 * KC), dtype=np.float16)
        w2h = np.zeros((128, CPC * HC), dtype=np.float16)
        for s in range(CPC):
            gc = comp_at[core][s]
            b1p[:, s * 2 * KC:(s + 1) * 2 * KC] = (
                (b1[gc] * WSCALE).reshape(2 * KC, 128).T.astype(np.float16)
            )
            w2h[:, HC * s:HC * (s + 1)] = (
                W2[gc, :, 0].reshape(HC, 128).T.astype(np.float16)
            )
        w10 = w1block(core, 0)  # [128, 2, KC, 512]
        head = np.concatenate(
            [
                b1p.view(np.uint8),
                w2h.view(np.uint8),
                zchunk(core, [0]).view(np.uint8),
                w10.reshape(128, 2 * KC * (H // 2)).view(np.uint8),
            ],
            axis=1,
        )
        w1rest = np.stack([w1block(core, s) for s in range(1, CPC)])
        in_maps.append(
            {
                "head": np.ascontiguousarray(head),
                "w1": np.ascontiguousarray(w1rest),
                "zb": np.ascontiguousarray(zchunk(core, [1, 2, 3])),
                "zc": np.ascontiguousarray(zchunk(core, [4, 5, 6, 7])),
            }
        )

    res = run_bass_kernel_spmd(nc, in_maps, list(range(NCORES)))

    out = np.zeros((B, O), dtype=np.float32)
    for core in range(NCORES):
        core_out = res.results[core]["out"].ravel()
        for s in range(CPC):
            gc = comp_at[core][s]
            ix = idx_by_company[gc]
            if len(ix):
                out[ix, 0] = core_out[ooffs[s]:ooffs[s] + len(ix)] + b2[gc, 0]
    return out
